# revision 1
# baseline (speedup 1.0000x reference)
"""GCN (4-layer, PyG-default GCNConv) forward on 8 Trainium2 NeuronCores.

Strategy (node-parallel / graph-parallel):
  - Nodes are partitioned contiguously across the 8 cores (1250 rows each,
    padded to 1280 = 10 blocks of 128).
  - Per layer: each core computes its row-slice of G = H @ W as a tiled PE
    GEMM (bf16 in / fp32 accumulate); the slices are AllGathered into a
    replicated HBM copy (one collective per layer — row-split variants were
    measured slower: the extra per-half chunk padding cost more than the
    comm/compute overlap saved).
  - Aggregation (symmetric-normalized adjacency including self-loops) runs
    per 128-destination-node block: source rows are fetched with dma_gather
    (256 indices per call — this ucode's limit — round-robined over 4 SWDGE
    queues so descriptor generation parallelizes across Q7 core pairs) and
    summed on the PE as OUT_block += S_chunk.T @ MSG_chunk, where S_chunk is
    a host-built dense [128-edge, 128-dst] matrix of edge norm weights.
  - Layer 4 is reassociated as (A_hat @ H4) @ W4 + b4 so the aggregation
    stays 512 wide and the tiny final GEMM runs in fp32.
  - log_softmax over the 2 classes is fused on-chip.
"""

import sys

sys.path.insert(0, "/opt/trn_rl_repo")

import numpy as np
import ml_dtypes

BF16 = ml_dtypes.bfloat16

# Problem constants (nn_GCN_39195871543847)
N, E, F_IN, HID, C = 10000, 160000, 2208, 512, 2
W_CORES = 8
RPC = N // W_CORES  # 1250 nodes per core
MB = 10  # 128-row blocks per core
RPAD = MB * 128  # 1280
HB_A = MB  # all blocks in one all-gather (splits cost more padding than
RH_A = HB_A * 128  # they save in overlap on this walrus/ncfw combination)
RH_B = 0
GH_A = W_CORES * RH_A
GH_B = 0
KFC = (F_IN + 127) // 128  # 18 contraction chunks for layer 1
KFP = KFC * 128  # 2304
C_PAD = 128  # pad 2 output classes to 128 bf16 (256B gather rows)
G_CHUNKS = 2  # 128-index chunks per dma_gather call (ucode limit: 256 idxs)
N_QUEUES = 4  # SWDGE queues for gather descriptor generation


def _install_drain_patch():
    """This container's walrus accepts at most one sync-wait per instruction;
    TileContext's final drain gets one wait per live semaphore. Split the
    extra waits onto single-wait NOPs."""
    import bass_rust
    import concourse.tile as tile
    from concourse.vector_clock import ScopedClock

    if getattr(tile.TileContext, "_drain_patch_installed", False):
        return

    def _drain_and_barrier(self, tick_clock, wait_clock):
        drain_inst = self.nc.sync.drain()
        wait_clock.add_sem_waits(
            drain_inst.ins, ScopedClock({None: tick_clock.global_clock})
        )
        si = drain_inst.ins.sync_info
        waits = list(si.on_wait or []) if si is not None else []
        if len(waits) > 1:
            si.on_wait = waits[:1]
            for w in waits[1:]:
                nop = self.nc.sync.nop(nofuse=True)
                nop.ins.sync_info = bass_rust.SyncInfo(on_wait=[w], on_update=[])
        self.nc.all_engine_barrier()
        assert self.sems is not None
        popped = self.nc._tile_sem_poison_stack.pop()
        assert popped is self._sem_poison
        self.nc.clear_and_free_semaphores(list(self.sems.allocated().values()))
        self.nc.all_engine_barrier()

    tile.TileContext._drain_and_barrier = _drain_and_barrier
    tile.TileContext._drain_patch_installed = True


# ----------------------------------------------------------------------------
# Host-side graph preprocessing
# ----------------------------------------------------------------------------


def _preprocess(edge_index):
    """Per core, per 128-dst block, split incoming edges by source half and
    build the S chunk stack plus the dma_gather index layout."""
    src = edge_index[0].astype(np.int64)
    dst = edge_index[1].astype(np.int64)
    loop = np.arange(N, dtype=np.int64)
    s = np.concatenate([src, loop])
    d = np.concatenate([dst, loop])
    deg = np.bincount(d, minlength=N).astype(np.float32)
    dinv = np.where(deg > 0, 1.0 / np.sqrt(deg), 0.0).astype(np.float32)
    norm = dinv[s] * dinv[d]

    core = d // RPC
    per_core = []
    ka = np.ones(MB, np.int64)  # per-block chunk count (max over cores)
    for c in range(W_CORES):
        m = core == c
        sc, dc, wc = s[m], d[m] - c * RPC, norm[m]
        s_core = sc // RPC
        s_loc = sc % RPC
        g_row = s_core * RH_A + s_loc
        blk = dc // 128
        order = np.argsort(blk, kind="stable")
        wc = wc[order]
        g_row, blk = g_row[order], blk[order]
        mloc = (dc[order]) % 128
        ca = np.bincount(blk, minlength=MB)
        ka = np.maximum(ka, (ca + 127) // 128)
        per_core.append((g_row, blk, mloc, wc, ca))

    tb_off = np.zeros(MB, np.int64)
    tb_off[1:] = np.cumsum(ka)[:-1]
    T = int(ka.sum())
    s_list, idx_list = [], []
    for g_row, blk, mloc, wc, ca in per_core:
        starts = np.zeros(MB, np.int64)
        starts[1:] = np.cumsum(ca)[:-1]
        pos = np.arange(len(g_row)) - starts[blk]
        t = tb_off[blk] + pos // 128
        k = pos % 128
        S = np.zeros((T, 128, 128), np.float32)
        S[t, k, mloc] = wc
        idx_flat = np.zeros(T * 128, np.int16)
        idx_flat[t * 128 + k] = g_row.astype(np.int16)
        lay16 = idx_flat.reshape(T * 8, 16).T  # [16, T*8]
        idx_list.append(np.tile(lay16, (8, 1)).astype(np.int16))
        # SBUF-resident layout [128 partitions(k), T, 128(m)]
        s_list.append(np.ascontiguousarray(S.transpose(1, 0, 2)).astype(BF16))
    return tuple(int(v) for v in ka), s_list, idx_list


def _prep_inputs(x, edge_index, W1, b1, W2, b2, W3, b3, W4, b4):
    ka, s_list, idx_list = _preprocess(edge_index)

    # xT per core: [MB, 128(p), KFC, 128(j)]; xT[m,p,k,j] = x[c*RPC+m*128+j, k*128+p]
    xts = []
    for c in range(W_CORES):
        xp = np.zeros((RPAD, KFP), np.float32)
        xp[:RPC, :F_IN] = x[c * RPC : (c + 1) * RPC]
        xt = xp.reshape(MB, 128, KFC, 128).transpose(0, 3, 2, 1)
        xts.append(np.ascontiguousarray(xt).astype(BF16))

    W1p = np.zeros((KFP, HID), np.float32)
    W1p[:F_IN] = W1
    W1l = np.ascontiguousarray(
        W1p.reshape(KFC, 128, HID).transpose(1, 0, 2)
    ).astype(BF16)
    W2l = np.ascontiguousarray(W2.reshape(4, 128, HID).transpose(1, 0, 2)).astype(BF16)
    W3l = np.ascontiguousarray(W3.reshape(4, 128, HID).transpose(1, 0, 2)).astype(BF16)
    W4p = np.zeros((HID, C_PAD), np.float32)
    W4p[:, :C] = W4
    W4l = np.ascontiguousarray(
        W4p.reshape(4, 128, C_PAD).transpose(1, 0, 2)
    ).astype(BF16)

    b1r = np.broadcast_to(b1, (128, HID)).astype(np.float32).copy()
    b2r = np.broadcast_to(b2, (128, HID)).astype(np.float32).copy()
    b3r = np.broadcast_to(b3, (128, HID)).astype(np.float32).copy()
    b4r = np.zeros((128, C_PAD), np.float32)
    b4r[:, :C] = b4

    in_maps = []
    for c in range(W_CORES):
        in_maps.append(
            {
                "xT": xts[c],
                "W1l": W1l, "W2l": W2l, "W3l": W3l, "W4l": W4l,
                "b1r": b1r, "b2r": b2r, "b3r": b3r, "b4r": b4r,
                "S_in": s_list[c],
                "idx_in": idx_list[c],
            }
        )
    return ka, in_maps


# ----------------------------------------------------------------------------
# Bass kernel builder
# ----------------------------------------------------------------------------

_cache = {}


def _build(ka):
    import concourse.bass as bass
    import concourse.mybir as mybir
    from concourse.bacc import Bacc
    from concourse.tile import TileContext
    from concourse.masks import make_identity

    f32 = mybir.dt.float32
    bf16 = mybir.dt.bfloat16
    i16 = mybir.dt.int16
    tb_off = [0] * MB
    for b in range(1, MB):
        tb_off[b] = tb_off[b - 1] + ka[b - 1]
    T = sum(ka)

    nc = Bacc(num_devices=W_CORES, num_swdge_queues=N_QUEUES)
    gq = [0]  # round-robin cursor over gather queues

    xT = nc.dram_tensor("xT", [MB, 128, KFC, 128], bf16, kind="ExternalInput")
    W1l = nc.dram_tensor("W1l", [128, KFC, HID], bf16, kind="ExternalInput")
    W2l = nc.dram_tensor("W2l", [128, 4, HID], bf16, kind="ExternalInput")
    W3l = nc.dram_tensor("W3l", [128, 4, HID], bf16, kind="ExternalInput")
    W4l = nc.dram_tensor("W4l", [128, 4, C_PAD], bf16, kind="ExternalInput")
    b1r = nc.dram_tensor("b1r", [128, HID], f32, kind="ExternalInput")
    b2r = nc.dram_tensor("b2r", [128, HID], f32, kind="ExternalInput")
    b3r = nc.dram_tensor("b3r", [128, HID], f32, kind="ExternalInput")
    b4r = nc.dram_tensor("b4r", [128, C_PAD], f32, kind="ExternalInput")
    S_in = nc.dram_tensor("S_in", [128, T, 128], bf16, kind="ExternalInput")
    idx_in = nc.dram_tensor("idx_in", [128, T * 8], i16, kind="ExternalInput")
    out = nc.dram_tensor("out", [RPAD, C], f32, kind="ExternalOutput")

    # per-layer bounce halves + gathered halves (layers 0..2 are G, 3 is H4)
    own_a, own_b, full_a, full_b = [], [], [], []
    l_wid = [HID, HID, HID, C_PAD]
    for l in range(4):
        own_a.append(
            nc.dram_tensor(f"own_a{l}", [RH_A, l_wid[l]], bf16, kind="Internal")
        )
        full_a.append(
            nc.dram_tensor(
                f"full_a{l}", [GH_A, l_wid[l]], bf16, kind="Internal",
                addr_space="Shared",
            )
        )
        if RH_B > 0:
            own_b.append(
                nc.dram_tensor(f"own_b{l}", [RH_B, HID], bf16, kind="Internal")
            )
            full_b.append(
                nc.dram_tensor(
                    f"full_b{l}", [GH_B, HID], bf16, kind="Internal",
                    addr_space="Shared",
                )
            )
        else:
            own_b.append(None)
            full_b.append(None)

    rg = [list(range(W_CORES))]

    with TileContext(nc) as tc:
        with (
            tc.tile_pool(name="const", bufs=1) as cpool,
            tc.tile_pool(name="work", bufs=2) as wpool,
            tc.tile_pool(name="psum", bufs=2, space="PSUM") as ppool,
        ):
            relu = mybir.ActivationFunctionType.Relu

            # cached index-count registers for dma_gather
            r_full = nc.gpsimd.to_reg(G_CHUNKS * 128)
            r_half = nc.gpsimd.to_reg(128)

            # ---- layer-1 GEMM inputs first (critical path) -------------------
            W1_sb = cpool.tile([128, KFC, HID], bf16)
            nc.sync.dma_start(out=W1_sb[:], in_=W1l[:])

            def allgather(own, full):
                nc.gpsimd.collective_compute(
                    "AllGather",
                    mybir.AluOpType.bypass,
                    ins=[own[:]],
                    outs=[full[:]],
                    replica_groups=rg,
                )

            def gemm_l1():
                for m in range(MB):
                    xm = wpool.tile([128, KFC, 128], bf16, tag="xm", bufs=3)
                    nc.sync.dma_start(out=xm[:], in_=xT[m])
                    ps = ppool.tile([128, HID], f32, tag="gps")
                    for k in range(KFC):
                        nc.tensor.matmul(
                            ps[:],
                            lhsT=xm[:, k, :],
                            rhs=W1_sb[:, k, :],
                            start=(k == 0),
                            stop=(k == KFC - 1),
                        )
                    gb = wpool.tile([128, HID], bf16, tag="gb", bufs=3)
                    nc.scalar.copy(gb[:], ps[:])
                    if m < HB_A:
                        nc.sync.dma_start(
                            out=own_a[0][m * 128 : (m + 1) * 128, :], in_=gb[:]
                        )
                        if m == HB_A - 1:
                            allgather(own_a[0], full_a[0])
                    else:
                        r0 = (m - HB_A) * 128
                        nc.sync.dma_start(
                            out=own_b[0][r0 : r0 + 128, :], in_=gb[:]
                        )
                        if m == MB - 1:
                            allgather(own_b[0], full_b[0])

            gemm_l1()

            # ---- remaining resident tensors (overlap the first collective) ---
            S_sb = cpool.tile([128, T, 128], bf16)
            nc.sync.dma_start(out=S_sb[:], in_=S_in[:])
            idx_sb = cpool.tile([128, T * 8], i16)
            nc.sync.dma_start(out=idx_sb[:], in_=idx_in[:])
            W2_sb = cpool.tile([128, 4, HID], bf16)
            nc.sync.dma_start(out=W2_sb[:], in_=W2l[:])
            W3_sb = cpool.tile([128, 4, HID], bf16)
            nc.sync.dma_start(out=W3_sb[:], in_=W3l[:])
            W4_sb = cpool.tile([128, 4, C_PAD], bf16)
            nc.sync.dma_start(out=W4_sb[:], in_=W4l[:])
            b_sb = []
            for nm, srcb in (("b1", b1r), ("b2", b2r), ("b3", b3r)):
                t = cpool.tile([128, HID], f32, tag=f"bias_{nm}")
                nc.sync.dma_start(out=t[:], in_=srcb[:])
                b_sb.append(t)
            b4_sb = cpool.tile([128, C_PAD], f32)
            nc.sync.dma_start(out=b4_sb[:], in_=b4r[:])
            id_bf = cpool.tile([128, 128], bf16)
            make_identity(nc, id_bf[:])
            id_f32 = cpool.tile([128, 128], f32)
            make_identity(nc, id_f32[:])

            def gather_chunks(ps, fsrc, t0, nk, first, last, w=HID):
                """Gather nk chunks of w-wide rows starting at chunk slot t0
                from fsrc and accumulate S.T @ MSG into ps."""
                for g0 in range(0, nk, G_CHUNKS):
                    ngc = min(G_CHUNKS, nk - g0)
                    tg = t0 + g0
                    msg = wpool.tile([128, G_CHUNKS, w], bf16, tag="msg", bufs=12)
                    nc.gpsimd.dma_gather(
                        out_ap=msg[:, :ngc, :],
                        in_ap=fsrc[:],
                        idxs_ap=idx_sb[:, tg * 8 : (tg + ngc) * 8],
                        num_idxs=ngc * 128,
                        num_idxs_reg=r_full if ngc == G_CHUNKS else r_half,
                        elem_size=w,
                        queue_num=gq[0],
                    )
                    gq[0] = (gq[0] + 1) % N_QUEUES
                    for u in range(ngc):
                        nc.tensor.matmul(
                            ps[:],
                            lhsT=S_sb[:, tg + u, :],
                            rhs=msg[:, u, :],
                            start=(first and g0 == 0 and u == 0),
                            stop=(last and g0 + u == nk - 1),
                        )

            def store_own(lslot, b, tile):
                """DMA a finished [128, HID] block to its bounce half and fire
                the half's AllGather when complete."""
                if b < HB_A:
                    nc.sync.dma_start(
                        out=own_a[lslot][b * 128 : (b + 1) * 128, :], in_=tile[:]
                    )
                    if b == HB_A - 1:
                        allgather(own_a[lslot], full_a[lslot])
                else:
                    r0 = (b - HB_A) * 128
                    nc.sync.dma_start(
                        out=own_b[lslot][r0 : r0 + 128, :], in_=tile[:]
                    )
                    if b == MB - 1:
                        allgather(own_b[lslot], full_b[lslot])

            def logsoftmax_block(ps, m):
                lg = wpool.tile([128, C_PAD], f32, tag="lg")
                nc.vector.tensor_add(out=lg[:], in0=ps[:], in1=b4_sb[:])
                mx = wpool.tile([128, 1], f32, tag="mx")
                nc.vector.tensor_reduce(
                    out=mx[:], in_=lg[:, :C], axis=mybir.AxisListType.X,
                    op=mybir.AluOpType.max,
                )
                t2 = wpool.tile([128, C], f32, tag="t2")
                nc.vector.tensor_scalar(
                    out=t2[:], in0=lg[:, :C], scalar1=mx[:], scalar2=None,
                    op0=mybir.AluOpType.subtract,
                )
                e2 = wpool.tile([128, C], f32, tag="e2")
                nc.scalar.activation(e2[:], t2[:], mybir.ActivationFunctionType.Exp)
                sm = wpool.tile([128, 1], f32, tag="sm")
                nc.vector.tensor_reduce(
                    out=sm[:], in_=e2[:], axis=mybir.AxisListType.X,
                    op=mybir.AluOpType.add,
                )
                ls = wpool.tile([128, 1], f32, tag="ls")
                nc.scalar.activation(ls[:], sm[:], mybir.ActivationFunctionType.Ln)
                o2 = wpool.tile([128, C], f32, tag="o2")
                nc.vector.tensor_scalar(
                    out=o2[:], in0=t2[:], scalar1=ls[:], scalar2=None,
                    op0=mybir.AluOpType.subtract,
                )
                nc.sync.dma_start(out=out[m * 128 : (m + 1) * 128, :], in_=o2[:])

            def layer(l, bias_t, mode):
                """One fused layer: per dst block, aggregate from the layer-l
                gathered halves, then immediately do this block's follow-up
                (next-layer GEMM / H4 store / final classifier) so the
                AllGathers fired mid-loop overlap the remaining blocks.
                mode: ("gemm", w_sb, lnext) | ("store_h",) | ("final",)"""
                for b in range(MB):
                    if mode[0] == "final":
                        # narrow aggregation of G4 = H4 @ W4, then classifier
                        ps = ppool.tile([128, C_PAD], f32, tag="aps", bufs=5)
                        gather_chunks(
                            ps, full_a[l], tb_off[b], ka[b], True, True, w=C_PAD
                        )
                        logsoftmax_block(ps, b)
                        continue
                    ps = ppool.tile([128, HID], f32, tag="aps", bufs=5)
                    gather_chunks(ps, full_a[l], tb_off[b], ka[b], True, True)
                    hf = wpool.tile([128, HID], f32, tag="hf", bufs=3)
                    nc.vector.tensor_add(out=hf[:], in0=ps[:], in1=bias_t[:])
                    hb = wpool.tile([128, HID], bf16, tag="hb", bufs=3)
                    nc.scalar.activation(hb[:], hf[:], relu)
                    # transpose into GEMM lhsT layout
                    ht = wpool.tile([128, 4, 128], bf16, tag="ht", bufs=4)
                    for g in range(4):
                        tp = ppool.tile([128, 128], bf16, tag="tps", bufs=1)
                        nc.tensor.transpose(
                            tp[:], hb[:, g * 128 : (g + 1) * 128], id_bf[:]
                        )
                        nc.vector.tensor_copy(out=ht[:, g, :], in_=tp[:])
                    _, w_sb, lnext = mode
                    wid = HID if lnext < 3 else C_PAD
                    gp = ppool.tile([128, wid], f32, tag="gps")
                    for k in range(4):
                        nc.tensor.matmul(
                            gp[:],
                            lhsT=ht[:, k, :],
                            rhs=w_sb[:, k, :],
                            start=(k == 0),
                            stop=(k == 3),
                        )
                    gb = wpool.tile([128, wid], bf16, tag="gb", bufs=3)
                    nc.scalar.copy(gb[:], gp[:])
                    store_own(lnext, b, gb)

            # ---- layers ----------------------------------------------------
            layer(0, b_sb[0], ("gemm", W2_sb, 1))
            layer(1, b_sb[1], ("gemm", W3_sb, 2))
            layer(2, b_sb[2], ("gemm", W4_sb, 3))
            layer(3, None, ("final",))

    nc.compile()
    return nc


# ----------------------------------------------------------------------------
# Entry point
# ----------------------------------------------------------------------------


def kernel(x, edge_index, batch, W1, b1, W2, b2, W3, b3, W4, b4, _trace=False):
    _install_drain_patch()
    from concourse.bass_utils import run_bass_kernel_spmd

    ka, in_maps = _prep_inputs(
        np.asarray(x, np.float32),
        np.asarray(edge_index),
        np.asarray(W1, np.float32), np.asarray(b1, np.float32),
        np.asarray(W2, np.float32), np.asarray(b2, np.float32),
        np.asarray(W3, np.float32), np.asarray(b3, np.float32),
        np.asarray(W4, np.float32), np.asarray(b4, np.float32),
    )
    key = tuple(ka)
    if key not in _cache:
        _cache[key] = _build(ka)
    nc = _cache[key]
    res = run_bass_kernel_spmd(
        nc, in_maps, core_ids=list(range(W_CORES)), trace=_trace
    )
    outp = np.concatenate(
        [res.results[c]["out"][:RPC] for c in range(W_CORES)], axis=0
    ).astype(np.float32)
    if _trace:
        return outp, res
    return outp

